# revision 1
# baseline (speedup 1.0000x reference)
"""Trainium2 Bass kernel for nn_MultiHeadAttention_81673098101666.

Reference computation (per batch b):
    qkv  = seq @ w_qkv.T ; q,k,v = split(qkv)        # seq [S,128], q/k/v [S,1024]
    scores = q @ k.T / 32 ; attn = softmax(scores)
    out  = attn @ v @ w_out.T + b_out                # [S, 128]

Key algebraic identity (INPUT_DIM=128 => rank-128 attention):
    scores^T = seq (Wk^T Wq) seq_q^T      with M  = Wk^T Wq   [128,128]
    out^T    = W2T^T (seq^T E^T) / sumexp with W2T = Wv^T Wout^T [128,128]
so the S^2-sized matmuls contract over 128 dims instead of 1024 (8x fewer
FLOPs) and Q/K/V are never materialized.

Sharding: 8 cores = 4 batches x 2 query-halves; no collectives. Each core
returns the unnormalized projected context (outT, [128, 1024]) plus the
softmax denominator; the host divides and adds the bias.

Device layouts (P=partition dim first):
    seqT  [128(i), 2048(k)]   seqq^T [128(i), 1024(q)]   seq_nat [k, i] tiles
    A = M^T-contracted seq: A[j, k] (lhsT for scores)
    ET[k, q] = exp(scoresT/32)
    C[i, q] = seq^T E^T ;  outT[c, q] = W2T^T C

All matmul operands are float32r (fp32 rounded to 11-bit mantissa, full-rate
PE path).
"""

import numpy as np

B, S, DIN = 4, 2048, 128
O = 1024
QPC = S // 2           # queries per core = 1024
QC = 512               # query-chunk width
N_CHUNK = QPC // QC    # 2
NKT = S // 128         # 16 key tiles
ND = O // 128          # 8 o-tiles (weight contraction)
SCALE = 1.0 / 32.0     # 1/sqrt(O)

_NC = None
PROFILE = False
LAST_RESULTS = None


def _body(ctx, tc, seqkv, seqn, seqq, Md, W2Td, outT_d, sumexp_d):
    import concourse.mybir as mybir

    nc = tc.nc
    f32 = mybir.dt.float32
    f32r = mybir.dt.float32r

    consts = ctx.enter_context(tc.tile_pool(name="consts", bufs=1))
    et_pool = ctx.enter_context(tc.tile_pool(name="et", bufs=16))
    c_pool = ctx.enter_context(tc.tile_pool(name="cp", bufs=2))
    out_pool = ctx.enter_context(tc.tile_pool(name="outs", bufs=4))
    psum = ctx.enter_context(tc.tile_pool(name="psum", bufs=1, space="PSUM"))

    # ---- loads: wave 1 = M/W2T + seqkv + seqq (feeds A -> scores -> exp);
    # seqn (only needed by C, ~10us later) queues behind on the same HW queues.
    M_sb = consts.tile([128, 128], f32r)       # M = Wk^T Wq (host-computed)
    nc.sync.dma_start(M_sb[:], Md[:].bitcast(f32r))
    W2T_sb = consts.tile([128, 128], f32r)     # W2T = Wv^T Wout^T (host)
    nc.sync.dma_start(W2T_sb[:], W2Td[:].bitcast(f32r))
    seqkv_sb = consts.tile([128, S], f32r)
    seqq_sb = consts.tile([128, QPC], f32r)
    for c in range(2):
        nc.sync.dma_start(seqkv_sb[:, c * 256:(c + 1) * 256],
                          seqkv[:, c * 256:(c + 1) * 256].bitcast(f32r))
    for c in range(4):
        nc.sync.dma_start(seqq_sb[:, c * 256:(c + 1) * 256],
                          seqq[:, c * 256:(c + 1) * 256].bitcast(f32r))
    for c in range(2, 8):
        nc.sync.dma_start(seqkv_sb[:, c * 256:(c + 1) * 256],
                          seqkv[:, c * 256:(c + 1) * 256].bitcast(f32r))
    seqn_sb = consts.tile([128, NKT * 128], f32r)  # seq natural [k, i], 16 tiles
    sn3 = seqn.bitcast(f32r).rearrange("(t p) i -> p t i", p=128)  # [128, 16, 128]
    snsb3 = seqn_sb[:].rearrange("p (t i) -> p t i", i=128)
    for t0, t1 in ((0, 4), (4, 8), (8, 12), (12, 16)):
        nc.sync.dma_start(snsb3[:, t0:t1, :], sn3[:, t0:t1, :])

    ones_f = consts.tile([128, 2], f32)
    nc.any.memset(ones_f[:], 1.0)
    ones_col = consts.tile([128, 2], f32r)
    nc.scalar.copy(ones_col[:], ones_f[:])

    A_sb = consts.tile([128, S], f32r)     # A[j, k]
    for ac in range(4):
        pa = psum.tile([128, 512], f32, tag="ctx", bufs=2)
        nc.tensor.matmul(pa[:], M_sb[:], seqkv_sb[:, ac * 512:(ac + 1) * 512],
                         start=True, stop=True)
        nc.vector.tensor_copy(A_sb[:, ac * 512:(ac + 1) * 512], pa[:])

    # ---- scores + exp for BOTH query chunks at once ---------------------
    ets = []
    for kt in range(NKT):
        pp = psum.tile([128, 1024], f32, tag="mm", bufs=2)
        for qc in range(N_CHUNK):
            nc.tensor.matmul(pp[:, qc * QC:(qc + 1) * QC],
                             A_sb[:, kt * 128:(kt + 1) * 128],
                             seqq_sb[:, qc * QC:(qc + 1) * QC],
                             start=True, stop=True, skip_group_check=True)
        et = et_pool.tile([128, 1024], f32r, tag="et")
        nc.scalar.activation(et[:], pp[:],
                             mybir.ActivationFunctionType.Exp, scale=float(SCALE))
        ets.append(et)

    # ---- C + sumexp accumulation for both chunks, interleaved per kt ----
    # (each kt-step consumes ets[kt] as soon as the exp chain produces it)
    pcs = []
    pses = []
    for qc in range(N_CHUNK):
        pcs.append(psum.tile([128, QC], f32, tag="ctx", bufs=2, name=f"pc{qc}"))
        pses.append(psum.tile([2, QC], f32, tag="aux", bufs=2, name=f"pse{qc}"))
    for kt in range(NKT):
        for qc in range(N_CHUNK):
            q0 = qc * QC
            nc.tensor.matmul(pcs[qc][:], seqn_sb[:, kt * 128:(kt + 1) * 128],
                             ets[kt][:, q0: q0 + QC],
                             start=(kt == 0), stop=(kt == NKT - 1))
            nc.tensor.matmul(pses[qc][:], ones_col[:], ets[kt][:, q0: q0 + QC],
                             start=(kt == 0), stop=(kt == NKT - 1))

    for qc in range(N_CHUNK):
        q0 = qc * QC
        C_sb = c_pool.tile([128, QC], f32r, tag="c")
        nc.vector.tensor_copy(C_sb[:], pcs[qc][:])
        se_sb = out_pool.tile([1, QC], f32, tag="se_sb")
        nc.vector.tensor_copy(se_sb[:], pses[qc][:1, :])
        nc.sync.dma_start(sumexp_d[:, q0: q0 + QC], se_sb[:])
        po = psum.tile([128, QC], f32, tag="mm", bufs=2)
        nc.tensor.matmul(po[:], W2T_sb[:], C_sb[:], start=True, stop=True)
        ot = out_pool.tile([128, QC], f32, tag="ot")
        nc.vector.tensor_copy(ot[:], po[:])
        nc.sync.dma_start(outT_d[:, q0: q0 + QC], ot[:])


def _build_nc():
    from contextlib import ExitStack

    import concourse.mybir as mybir
    import concourse.tile as tile
    from concourse import bacc

    f32 = mybir.dt.float32
    nc = bacc.Bacc("TRN2", target_bir_lowering=False, debug=False, num_devices=8)
    seqkv = nc.dram_tensor("seqT_kv", [128, S], f32, kind="ExternalInput").ap()
    seqn = nc.dram_tensor("seq_nat", [S, 128], f32, kind="ExternalInput").ap()
    seqq = nc.dram_tensor("seqT_q", [128, QPC], f32, kind="ExternalInput").ap()
    Md = nc.dram_tensor("M_in", [128, 128], f32, kind="ExternalInput").ap()
    W2Td = nc.dram_tensor("W2T_in", [128, 128], f32, kind="ExternalInput").ap()
    outT_d = nc.dram_tensor("outT", [128, QPC], f32, kind="ExternalOutput").ap()
    sumexp_d = nc.dram_tensor("sumexp", [1, QPC], f32, kind="ExternalOutput").ap()

    with tile.TileContext(nc) as tc:
        with ExitStack() as ctx:
            _body(ctx, tc, seqkv, seqn, seqq, Md, W2Td, outT_d, sumexp_d)
    nc.compile()
    return nc


def get_nc():
    global _NC
    if _NC is None:
        _NC = _build_nc()
    return _NC


def make_in_maps(sequence, w_qkv, w_out):
    seqT = np.ascontiguousarray(np.transpose(sequence, (0, 2, 1)))  # [B, 128, S]
    wq, wk, wv = w_qkv[:O], w_qkv[O:2 * O], w_qkv[2 * O:]
    M = np.ascontiguousarray(wk.T @ wq)            # [128, 128]
    W2T = np.ascontiguousarray(wv.T @ w_out.T)     # [128, 128]
    in_maps = []
    for c in range(8):
        b, h = c // 2, c % 2
        in_maps.append({
            "seqT_kv": seqT[b],
            "seq_nat": np.ascontiguousarray(sequence[b]),
            "seqT_q": np.ascontiguousarray(seqT[b][:, h * QPC:(h + 1) * QPC]),
            "M_in": M,
            "W2T_in": W2T,
        })
    return in_maps


def kernel(sequence, w_qkv, w_out, b_out):
    global LAST_RESULTS
    from concourse.bass_utils import run_bass_kernel_spmd

    sequence = np.asarray(sequence, dtype=np.float32)
    w_qkv = np.asarray(w_qkv, dtype=np.float32)
    w_out = np.asarray(w_out, dtype=np.float32)
    b_out = np.asarray(b_out, dtype=np.float32)

    nc = get_nc()
    in_maps = make_in_maps(sequence, w_qkv, w_out)
    kw = {}
    if PROFILE:
        kw = dict(trace=True, trace_cores=[0])
    res = run_bass_kernel_spmd(nc, in_maps, list(range(8)), **kw)
    LAST_RESULTS = res

    out = np.empty((B, S, DIN), np.float32)
    for c in range(8):
        b, h = c // 2, c % 2
        outT = res.results[c]["outT"]          # [128, 1024] unnormalized c-major
        se = res.results[c]["sumexp"][0]       # [1024]
        out[b, h * QPC:(h + 1) * QPC, :] = outT.T / se[:, None] + b_out[None, :]
    return out



# revision 4
# speedup vs baseline: 1.1134x; 1.1134x over previous
"""Trainium2 Bass kernel for nn_MultiHeadAttention_81673098101666.

Reference computation (per batch b):
    qkv  = seq @ w_qkv.T ; q,k,v = split(qkv)        # seq [S,128], q/k/v [S,1024]
    scores = q @ k.T / 32 ; attn = softmax(scores)
    out  = attn @ v @ w_out.T + b_out                # [S, 128]

Key algebraic identity (INPUT_DIM=128 => rank-128 attention):
    scores^T = seq (Wk^T Wq) seq_q^T      with M  = Wk^T Wq   [128,128]
    out^T    = W2T^T (seq^T E^T) / sumexp with W2T = Wv^T Wout^T [128,128]
so the S^2-sized matmuls contract over 128 dims instead of 1024 (8x fewer
FLOPs) and Q/K/V are never materialized.

Sharding: 8 cores = 4 batches x 2 query-halves; no collectives. Each core
returns the unnormalized projected context (outT, [128, 1024]) plus the
softmax denominator; the host divides and adds the bias.

v2 (all fp16 on device):
  - fp16 operands everywhere: halves DMA bytes, enables FWL weight loads,
    2x DVE modes; psum stays f32.
  - sumexp off the PE: two DVE accumulation chains over the ET tiles
    (even/odd kt) + 4 single-partition matmuls against a ones column.
  - DMA issues split across sync/scalar/gpsimd queues (HWDGE issue costs
    ~650ns serialized per queue).
  - PE warmup matmuls during the DMA wait so HAM un-throttles (1.2->2.4GHz)
    before the real matmuls arrive.
  - fp16 outputs, host upcasts/normalizes.
"""

import numpy as np

B, S, DIN = 4, 2048, 128
O = 1024
QPC = S // 2           # queries per core = 1024
NKT = S // 128         # 16 key tiles
SCALE = 1.0 / 32.0     # 1/sqrt(O)

_NC = None
PROFILE = False
LAST_RESULTS = None


def _body(ctx, tc, seqkv, seqn, seqq, MW, outT_d, sumexp_d):
    import concourse.mybir as mybir

    nc = tc.nc
    f32 = mybir.dt.float32
    f16 = mybir.dt.float16
    AF = mybir.ActivationFunctionType

    consts = ctx.enter_context(tc.tile_pool(name="consts", bufs=1))
    et_pool = ctx.enter_context(tc.tile_pool(name="et", bufs=6))
    acc_pool = ctx.enter_context(tc.tile_pool(name="acc", bufs=6))
    out_pool = ctx.enter_context(tc.tile_pool(name="outs", bufs=4))
    psum = ctx.enter_context(tc.tile_pool(name="psum", bufs=1, space="PSUM"))

    warm = consts.tile([128, 512], f16)
    ones = consts.tile([128, 2], f16)
    nc.vector.memset(warm[:], 0.0)
    nc.vector.memset(ones[:], 1.0)

    MW_sb = consts.tile([128, 256], f16)     # cols 0:128 = M, 128:256 = W2T
    seqkv_sb = consts.tile([128, S], f16)
    seqq_sb = consts.tile([128, QPC], f16)
    seqn_sb = consts.tile([128, S], f16)     # seq natural [k,i], 16 tiles on free dim
    A_sb = consts.tile([128, S], f16)        # A[j, k] (lhsT for scores)

    # ---- DMA issues. gpsimd = SWDGE queue (engine otherwise idle);
    # sync = HWDGE; ACT queue kept free for the activation table load.
    nc.gpsimd.dma_start(MW_sb[:], MW[:])
    sn3 = seqn.rearrange("(t p) i -> p t i", p=128)       # [128, 16, 128]
    snsb3 = seqn_sb[:].rearrange("p (t i) -> p t i", i=128)
    nc.gpsimd.dma_start(snsb3[:, 0:4, :], sn3[:, 0:4, :])
    nc.gpsimd.dma_start(snsb3[:, 4:16, :], sn3[:, 4:16, :])
    nc.sync.dma_start(seqkv_sb[:, 0:512], seqkv[:, 0:512])
    nc.sync.dma_start(seqq_sb[:], seqq[:])
    nc.sync.dma_start(seqkv_sb[:, 512:1024], seqkv[:, 512:1024])
    nc.sync.dma_start(seqkv_sb[:, 1024:2048], seqkv[:, 1024:2048])

    # ---- PE warmup: wake HAM out of 4/8 clock gating while DMAs land.
    pw = psum.tile([128, 1024], f32, tag="mm", bufs=2, name="warm")
    for _ in range(4):
        nc.tensor.matmul(pw[:, :512], warm[:, :128], warm[:],
                         start=True, stop=True, skip_group_check=True)

    # ---- A chunks (A[:, 512c:512c+512] = M^T-contraction of seqkv chunk c)
    def emit_A(c):
        pa = psum.tile([128, 512], f32, tag="pa", bufs=2, name=f"pa{c}")
        nc.tensor.matmul(pa[:], MW_sb[:, :128],
                         seqkv_sb[:, c * 512:(c + 1) * 512],
                         start=True, stop=True)
        nc.vector.tensor_copy(A_sb[:, c * 512:(c + 1) * 512], pa[:])

    # pc: C accumulator over all kt (two interleaved psum groups, one per half)
    pc = psum.tile([128, 1024], f32, tag="ctx", bufs=1, name="pc")

    acc = {0: None, 1: None}   # even / odd kt accumulation chains
    ets = []

    emit_A(0)
    for kt in range(NKT):
        if kt == 2:
            emit_A(1)
        elif kt == 4:
            emit_A(2)
        elif kt == 6:
            emit_A(3)
        # scores^T[k, q] for this key tile (both q halves)
        pp = psum.tile([128, 1024], f32, tag="mm", bufs=2, name=f"pp{kt}")
        for h in range(2):
            nc.tensor.matmul(pp[:, h * 512:(h + 1) * 512],
                             A_sb[:, kt * 128:(kt + 1) * 128],
                             seqq_sb[:, h * 512:(h + 1) * 512],
                             start=True, stop=True, skip_group_check=True)
        et = et_pool.tile([128, 1024], f16, tag="et")
        nc.scalar.activation(et[:], pp[:], AF.Exp, scale=float(SCALE))
        ets.append(et)
        # C accumulation: pc[i, q] += seqn_tile^T-contract et
        for h in range(2):
            nc.tensor.matmul(pc[:, h * 512:(h + 1) * 512],
                             seqn_sb[:, kt * 128:(kt + 1) * 128],
                             et[:, h * 512:(h + 1) * 512],
                             start=(kt == 0), stop=(kt == NKT - 1),
                             skip_group_check=True)
        # sumexp partial accumulation on DVE (parity-split chains)
        par = kt % 2
        if kt >= 2:
            prev = acc[par] if acc[par] is not None else ets[par]
            na = acc_pool.tile([128, 1024], f16, tag="acc")
            nc.vector.tensor_add(na[:], prev[:], et[:])
            acc[par] = na

    # ---- sumexp: reduce the two chain results over partitions via ones-matmul
    se_sb = out_pool.tile([1, QPC], f16, tag="se_sb")
    for h in range(2):
        pse = psum.tile([128, 512], f32, tag="pa", bufs=2, name=f"pse{h}")
        nc.tensor.matmul(pse[:1, :], ones[:, :1],
                         acc[0][:, h * 512:(h + 1) * 512],
                         start=True, stop=False, skip_group_check=True)
        nc.tensor.matmul(pse[:1, :], ones[:, :1],
                         acc[1][:, h * 512:(h + 1) * 512],
                         start=False, stop=True, skip_group_check=True)
        nc.vector.tensor_copy(se_sb[:, h * 512:(h + 1) * 512], pse[:1, :])
    nc.gpsimd.dma_start(sumexp_d[:], se_sb[:])

    # ---- output projection per half: outT = W2T^T C
    C_sb = out_pool.tile([128, QPC], f16, tag="c")
    for h in range(2):
        nc.vector.tensor_copy(C_sb[:, h * 512:(h + 1) * 512],
                              pc[:, h * 512:(h + 1) * 512])
        po = psum.tile([128, 1024], f32, tag="mm", bufs=2, name=f"po{h}")
        nc.tensor.matmul(po[:, :512], MW_sb[:, 128:256],
                         C_sb[:, h * 512:(h + 1) * 512],
                         start=True, stop=True, skip_group_check=True)
        ot = out_pool.tile([128, 512], f16, tag="ot")
        nc.vector.tensor_copy(ot[:], po[:, :512])
        nc.sync.dma_start(outT_d[:, h * 512:(h + 1) * 512], ot[:])


def _build_nc():
    from contextlib import ExitStack

    import concourse.mybir as mybir
    import concourse.tile as tile
    from concourse import bacc

    f32 = mybir.dt.float32
    f16 = mybir.dt.float16
    nc = bacc.Bacc("TRN2", target_bir_lowering=False, debug=False, num_devices=8)
    seqkv = nc.dram_tensor("seqT_kv", [128, S], f16, kind="ExternalInput").ap()
    seqn = nc.dram_tensor("seq_nat", [S, 128], f16, kind="ExternalInput").ap()
    seqq = nc.dram_tensor("seqT_q", [128, QPC], f16, kind="ExternalInput").ap()
    MW = nc.dram_tensor("MW_in", [128, 256], f16, kind="ExternalInput").ap()
    outT_d = nc.dram_tensor("outT", [128, QPC], f16, kind="ExternalOutput").ap()
    sumexp_d = nc.dram_tensor("sumexp", [1, QPC], f16, kind="ExternalOutput").ap()

    with tile.TileContext(nc) as tc:
        with ExitStack() as ctx:
            _body(ctx, tc, seqkv, seqn, seqq, MW, outT_d, sumexp_d)
    nc.compile()
    return nc


def get_nc():
    global _NC
    if _NC is None:
        _NC = _build_nc()
    return _NC


def make_in_maps(sequence, w_qkv, w_out):
    seq16 = sequence.astype(np.float16)                       # [B, S, 128]
    seqT16 = np.ascontiguousarray(seq16.transpose(0, 2, 1))   # [B, 128, S]
    wq, wk, wv = w_qkv[:O], w_qkv[O:2 * O], w_qkv[2 * O:]
    M = (wk.T @ wq).astype(np.float16)            # [128, 128]
    W2T = (wv.T @ w_out.T).astype(np.float16)     # [128, 128]
    MW = np.ascontiguousarray(np.concatenate([M, W2T], axis=1))
    in_maps = []
    for c in range(8):
        b, h = c // 2, c % 2
        in_maps.append({
            "seqT_kv": seqT16[b],
            "seq_nat": np.ascontiguousarray(seq16[b]),
            "seqT_q": np.ascontiguousarray(seqT16[b][:, h * QPC:(h + 1) * QPC]),
            "MW_in": MW,
        })
    return in_maps


def kernel(sequence, w_qkv, w_out, b_out):
    global LAST_RESULTS
    from concourse.bass_utils import run_bass_kernel_spmd

    sequence = np.asarray(sequence, dtype=np.float32)
    w_qkv = np.asarray(w_qkv, dtype=np.float32)
    w_out = np.asarray(w_out, dtype=np.float32)
    b_out = np.asarray(b_out, dtype=np.float32)

    nc = get_nc()
    in_maps = make_in_maps(sequence, w_qkv, w_out)
    kw = {}
    if PROFILE:
        kw = dict(trace=True, trace_cores=[0])
    res = run_bass_kernel_spmd(nc, in_maps, list(range(8)), **kw)
    LAST_RESULTS = res

    out = np.empty((B, S, DIN), np.float32)
    for c in range(8):
        b, h = c // 2, c % 2
        outT = res.results[c]["outT"].astype(np.float32)       # [128, 1024]
        se = res.results[c]["sumexp"].astype(np.float32)[0]    # [1024]
        out[b, h * QPC:(h + 1) * QPC, :] = outT.T / se[:, None] + b_out[None, :]
    return out
